# revision 1
# baseline (speedup 1.0000x reference)
"""Single transformer encoder layer on 8 Trainium2 NeuronCores.

Sharding: token-data-parallel. Each core owns 512 query tokens (4 cores
per batch element) and computes the K/V projections for its whole batch
(duplicated across the 4 cores sharing a batch — buys zero cross-core
communication), then attention, Wo, LN1, FFN, LN2 fully locally.

All activations live in transposed [feature, token] layout on-chip so
every GEMM is lhsT.T @ rhs with contraction on the partition dim, and
LayerNorm stats are computed with ones-matmuls (fp32) — no on-chip
transposes anywhere.  Matmul inputs bf16 (fp32 PSUM accumulation);
softmax skips max-subtraction (scores are ~N(0, 0.4^2), exp is safe);
the softmax denominator comes free from a ones-column appended to V.
"""

import sys

sys.path.insert(0, "/opt/trn_rl_repo")

import numpy as np
import ml_dtypes
from contextlib import ExitStack

import concourse.bass as bass
import concourse.mybir as mybir
import concourse.tile as tile
from concourse import bacc
import concourse.bass_utils as bass_utils

F32 = mybir.dt.float32
BF16 = mybir.dt.bfloat16
AF = mybir.ActivationFunctionType
ALU = mybir.AluOpType

B, S, D = 2, 2048, 1024
H, DK, DV, DFF = 16, 64, 64, 4096
EPS = 1e-5
NCORES = 8
TOK = 512          # query tokens per core
SB = 2048          # batch tokens (K/V length)
NKC = SB // 128    # 16 key chunks
NDC = D // 128     # 8 feature chunks
NFC = DFF // 128   # 32 ffn chunks


def _bf(x):
    return np.ascontiguousarray(x.astype(ml_dtypes.bfloat16))


def _f32(x):
    return np.ascontiguousarray(x.astype(np.float32))


def _dram_chunked(t, ncols):
    """View a [R, ncols] DRAM tensor as [128, R//128, ncols]."""
    return t[:].rearrange("(c p) n -> p c n", p=128)


def build():
    nc = bacc.Bacc(name="encoder_layer", num_devices=NCORES)

    # ---- DRAM I/O ----
    xkT = nc.dram_tensor("xkT", [D, SB], BF16, kind="ExternalInput")
    xqT = nc.dram_tensor("xqT", [D, TOK], BF16, kind="ExternalInput")
    xqTf = nc.dram_tensor("xqTf", [D, TOK], F32, kind="ExternalInput")
    wqkv = nc.dram_tensor("wqkv", [D, 3 * D], BF16, kind="ExternalInput")
    bqkv = nc.dram_tensor("bqkv", [3 * D, 1], F32, kind="ExternalInput")
    bv_row = nc.dram_tensor("bv_row", [1, D], BF16, kind="ExternalInput")
    wo = nc.dram_tensor("wo", [D, D], BF16, kind="ExternalInput")
    bo = nc.dram_tensor("bo", [D, 1], F32, kind="ExternalInput")
    w1 = nc.dram_tensor("w1", [D, DFF], BF16, kind="ExternalInput")
    b1 = nc.dram_tensor("b1", [DFF, 1], F32, kind="ExternalInput")
    w2 = nc.dram_tensor("w2", [DFF, D], BF16, kind="ExternalInput")
    b2 = nc.dram_tensor("b2", [D, 1], F32, kind="ExternalInput")
    g1 = nc.dram_tensor("g1", [D, 1], F32, kind="ExternalInput")
    be1 = nc.dram_tensor("be1", [D, 1], F32, kind="ExternalInput")
    g2 = nc.dram_tensor("g2", [D, 1], F32, kind="ExternalInput")
    be2 = nc.dram_tensor("be2", [D, 1], F32, kind="ExternalInput")
    outT = nc.dram_tensor("outT", [D, TOK], F32, kind="ExternalOutput")

    with tile.TileContext(nc) as tc, ExitStack() as top:
        sp = top.enter_context(tc.tile_pool(name="smalls", bufs=1))

        # small constants
        ones_c = sp.tile([128, 1], F32)          # ones column (lhsT for col-sums)
        nc.vector.memset(ones_c, 1.0)
        ones_r = sp.tile([1, 128], F32)          # ones row (lhsT for broadcasts)
        nc.vector.memset(ones_r, 1.0)
        eps_t = sp.tile([1, 1], F32)
        nc.vector.memset(eps_t, EPS)
        # tensors that live until the FFN (mid) and attention-only (attn)
        mid = top.enter_context(tc.tile_pool(name="mid", bufs=1))
        CT_sb = mid.tile([128, NDC, TOK], BF16)        # ctx^T [hv_all, 512]
        xres_sb = mid.tile([128, NDC, TOK], F32)       # x residual (fp32)
        ln1_sb = mid.tile([128, NDC, TOK], F32)        # LN1 out fp32 (residual 2)
        ln1b_sb = mid.tile([128, NDC, TOK], BF16)      # LN1 out bf16 (FFN rhs)

        attn_cm = tc.tile_pool(name="attn", bufs=1)
        attn = attn_cm.__enter__()
        QT_sb = attn.tile([128, NDC, TOK], BF16)       # Q^T  [dk_all, 512]
        KT_sb = attn.tile([128, NDC, SB], BF16)        # K^T  [dk_all, 2048]
        V_sb = attn.tile([128, NKC, H, DV + 1], BF16)  # V natural + ones col

        bqkv_sb = sp.tile([128, 24], F32)
        nc.sync.dma_start(out=bqkv_sb, in_=_dram_chunked(bqkv, 1).rearrange("p c n -> p (c n)"))
        bv_bc = sp.tile([128, D], BF16)          # bv broadcast across partitions
        nc.sync.dma_start(out=bv_bc, in_=bv_row[:].to_broadcast([128, D]))
        # ---------------- Phase 1: QKV projections ----------------
        with ExitStack() as ph1:
            xpool = ph1.enter_context(tc.tile_pool(name="xin", bufs=1))
            wpool = ph1.enter_context(tc.tile_pool(name="wq", bufs=4))
            pq = ph1.enter_context(tc.tile_pool(name="pq", bufs=4, space="PSUM"))

            xq_sb = xpool.tile([128, NDC, TOK], BF16)
            nc.sync.dma_start(out=xq_sb, in_=_dram_chunked(xqT, TOK))
            wv_sb = xpool.tile([128, NDC, D], BF16)
            nc.sync.dma_start(out=wv_sb, in_=_dram_chunked(wqkv, 3 * D)[:, :, 2 * D:3 * D])
            xk_sb = xpool.tile([128, NDC, SB], BF16)
            for dc in range(NDC):
                nc.sync.dma_start(out=xk_sb[:, dc, :],
                                  in_=_dram_chunked(xkT, SB)[:, dc, :])
            nc.vector.memset(V_sb[:, :, :, DV:DV + 1], 1.0)

            # Q^T (heads stacked on the M axis)
            for mc in range(NDC):
                wq_t = wpool.tile([128, NDC, 128], BF16, tag="wq")
                nc.sync.dma_start(
                    out=wq_t, in_=_dram_chunked(wqkv, 3 * D)[:, :, mc * 128:(mc + 1) * 128])
                ps = pq.tile([128, TOK], F32, tag="ps")
                for dc in range(NDC):
                    nc.tensor.matmul(ps, wq_t[:, dc, :], xq_sb[:, dc, :],
                                     start=(dc == 0), stop=(dc == NDC - 1))
                nc.vector.tensor_scalar(out=QT_sb[:, mc, :], in0=ps,
                                        scalar1=bqkv_sb[:, mc:mc + 1], scalar2=None,
                                        op0=ALU.add)

            # V natural: lhsT = x^T chunk (stationary), rhs = Wv slice
            for tc_ in range(NKC):
                ps0 = pq.tile([128, TOK], F32, tag="ps")
                ps1 = pq.tile([128, TOK], F32, tag="ps")
                for dc in range(NDC):
                    lhs = xk_sb[:, dc, tc_ * 128:(tc_ + 1) * 128]
                    nc.tensor.matmul(ps0, lhs, wv_sb[:, dc, 0:512],
                                     start=(dc == 0), stop=(dc == NDC - 1))
                    nc.tensor.matmul(ps1, lhs, wv_sb[:, dc, 512:1024],
                                     start=(dc == 0), stop=(dc == NDC - 1))
                nc.vector.tensor_tensor(
                    out=V_sb[:, tc_, 0:8, 0:DV],
                    in0=ps0[:].rearrange("p (h j) -> p h j", j=DV),
                    in1=bv_bc[:, 0:512].rearrange("p (h j) -> p h j", j=DV),
                    op=ALU.add)
                nc.vector.tensor_tensor(
                    out=V_sb[:, tc_, 8:16, 0:DV],
                    in0=ps1[:].rearrange("p (h j) -> p h j", j=DV),
                    in1=bv_bc[:, 512:1024].rearrange("p (h j) -> p h j", j=DV),
                    op=ALU.add)

            # K^T
            for mc in range(NDC):
                wk_t = wpool.tile([128, NDC, 128], BF16, tag="wq")
                nc.sync.dma_start(
                    out=wk_t,
                    in_=_dram_chunked(wqkv, 3 * D)[:, :, D + mc * 128:D + (mc + 1) * 128])
                for tt in range(SB // TOK):
                    ps = pq.tile([128, TOK], F32, tag="ps")
                    for dc in range(NDC):
                        nc.tensor.matmul(ps, wk_t[:, dc, :],
                                         xk_sb[:, dc, tt * TOK:(tt + 1) * TOK],
                                         start=(dc == 0), stop=(dc == NDC - 1))
                    nc.vector.tensor_scalar(out=KT_sb[:, mc, tt * TOK:(tt + 1) * TOK],
                                            in0=ps, scalar1=bqkv_sb[:, 8 + mc:9 + mc],
                                            scalar2=None, op0=ALU.add)

        bo_sb = sp.tile([128, 8], F32)
        nc.sync.dma_start(out=bo_sb, in_=_dram_chunked(bo, 1).rearrange("p c n -> p (c n)"))
        b1_sb = sp.tile([128, 32], F32)
        nc.sync.dma_start(out=b1_sb, in_=_dram_chunked(b1, 1).rearrange("p c n -> p (c n)"))
        b2_sb = sp.tile([128, 8], F32)
        nc.sync.dma_start(out=b2_sb, in_=_dram_chunked(b2, 1).rearrange("p c n -> p (c n)"))
        g1_sb = sp.tile([128, 8], F32)
        nc.sync.dma_start(out=g1_sb, in_=_dram_chunked(g1, 1).rearrange("p c n -> p (c n)"))
        be1_sb = sp.tile([128, 8], F32)
        nc.sync.dma_start(out=be1_sb, in_=_dram_chunked(be1, 1).rearrange("p c n -> p (c n)"))
        g2_sb = sp.tile([128, 8], F32)
        nc.sync.dma_start(out=g2_sb, in_=_dram_chunked(g2, 1).rearrange("p c n -> p (c n)"))
        be2_sb = sp.tile([128, 8], F32)
        nc.sync.dma_start(out=be2_sb, in_=_dram_chunked(be2, 1).rearrange("p c n -> p (c n)"))

        nc.sync.dma_start(out=xres_sb, in_=_dram_chunked(xqTf, TOK))

        # ---------------- Phase 2: attention ----------------
        # Head pairs (2h, 2h+1) live in halves of the same KT/QT chunk, so
        # their QK^T matmuls row-pack onto disjoint halves of the PE array
        # and run concurrently.  Scores for 2 key-chunks (KC2=256 keys) are
        # batched into one [128, 1024] PSUM tile so each Exp activation
        # covers 1024 elements per lane.
        with ExitStack() as ph2:
            psS = ph2.enter_context(tc.tile_pool(name="psS", bufs=3, space="PSUM"))
            psC = ph2.enter_context(tc.tile_pool(name="psC", bufs=2, space="PSUM"))
            apool = ph2.enter_context(tc.tile_pool(name="apool", bufs=4))
            npool = ph2.enter_context(tc.tile_pool(name="npool", bufs=2))

            for hp in range(H // 2):
                ctx0 = psC.tile([DV + 1, TOK], F32, tag="ctx")
                ctx1 = psC.tile([DV + 1, TOK], F32, tag="ctx")
                for kp in range(NKC // 2):
                    s0 = psS.tile([128, 2 * TOK], F32, tag="s")
                    s1 = psS.tile([128, 2 * TOK], F32, tag="s")
                    for j in range(2):
                        kc = 2 * kp + j
                        ksl = slice(kc * 128, (kc + 1) * 128)
                        osl = slice(j * TOK, (j + 1) * TOK)
                        nc.tensor.matmul(s0[:, osl],
                                         KT_sb[0:64, hp, ksl], QT_sb[0:64, hp, :],
                                         start=True, stop=True)
                        nc.tensor.matmul(s1[:, osl],
                                         KT_sb[64:128, hp, ksl], QT_sb[64:128, hp, :],
                                         start=True, stop=True)
                    a0 = apool.tile([128, 2 * TOK], BF16, tag="a")
                    a1 = apool.tile([128, 2 * TOK], BF16, tag="a")
                    nc.scalar.activation(out=a0, in_=s0, func=AF.Exp,
                                         scale=1.0 / np.sqrt(DK))
                    nc.scalar.activation(out=a1, in_=s1, func=AF.Exp,
                                         scale=1.0 / np.sqrt(DK))
                    for j in range(2):
                        kc = 2 * kp + j
                        osl = slice(j * TOK, (j + 1) * TOK)
                        nc.tensor.matmul(ctx0, V_sb[:, kc, 2 * hp, :], a0[:, osl],
                                         start=(kc == 0), stop=(kc == NKC - 1))
                        nc.tensor.matmul(ctx1, V_sb[:, kc, 2 * hp + 1, :], a1[:, osl],
                                         start=(kc == 0), stop=(kc == NKC - 1))
                # softmax normalization: r = 1/denominator, broadcast, multiply
                for j, ctx_ps in ((0, ctx0), (1, ctx1)):
                    off = j * 64
                    d_t = npool.tile([1, TOK], F32, tag="d")
                    nc.vector.tensor_copy(out=d_t, in_=ctx_ps[DV:DV + 1, :])
                    r_t = npool.tile([1, TOK], F32, tag="r")
                    nc.vector.reciprocal_approx_fast(out=r_t, in_=d_t)
                    rb_t = npool.tile([64, TOK], F32, tag="rb")
                    nc.gpsimd.partition_broadcast(rb_t[:], r_t[:], channels=64)
                    nc.vector.tensor_tensor(out=CT_sb[off:off + 64, hp, :],
                                            in0=ctx_ps[0:DV, :], in1=rb_t, op=ALU.mult)

        attn_cm.__exit__(None, None, None)  # free QT/KT/V before the FFN

        # ---------------- Phase 3: Wo + residual + LN1 ----------------
        with ExitStack() as ph3:
            wopool = ph3.enter_context(tc.tile_pool(name="wopool", bufs=1))
            psO = ph3.enter_context(tc.tile_pool(name="psO", bufs=2, space="PSUM"))
            psSt = ph3.enter_context(tc.tile_pool(name="psSt", bufs=1, space="PSUM"))
            lnp = ph3.enter_context(tc.tile_pool(name="lnp", bufs=2))
            y1_sb = wopool.tile([128, NDC, TOK], F32)
            sq_sb = wopool.tile([128, NDC, TOK], F32)

            wo_sb = wopool.tile([128, NDC, D], BF16)
            nc.sync.dma_start(out=wo_sb, in_=_dram_chunked(wo, D))
            for mc in range(NDC):
                ps = psO.tile([128, TOK], F32, tag="o")
                for hc in range(NDC):
                    nc.tensor.matmul(ps, wo_sb[:, hc, mc * 128:(mc + 1) * 128],
                                     CT_sb[:, hc, :],
                                     start=(hc == 0), stop=(hc == NDC - 1))
                # y1 = (mha + bo) + x
                nc.vector.scalar_tensor_tensor(out=y1_sb[:, mc, :], in0=ps,
                                               scalar=bo_sb[:, mc:mc + 1],
                                               in1=xres_sb[:, mc, :],
                                               op0=ALU.add, op1=ALU.add)

            _layernorm(nc, tc, psSt, lnp, y1_sb, sq_sb, ones_c, ones_r, eps_t,
                       g1_sb, be1_sb, ln1_sb, ln1b_sb)

        # ---------------- Phase 4: FFN + residual + LN2 ----------------
        with ExitStack() as ph4:
            w1pool = ph4.enter_context(tc.tile_pool(name="w1pool", bufs=3))
            w2pool = ph4.enter_context(tc.tile_pool(name="w2pool", bufs=2))
            hpool = ph4.enter_context(tc.tile_pool(name="hpool", bufs=1))
            psF = ph4.enter_context(tc.tile_pool(name="psF", bufs=3, space="PSUM"))
            psF2 = ph4.enter_context(tc.tile_pool(name="psF2", bufs=3, space="PSUM"))
            psSt = ph4.enter_context(tc.tile_pool(name="psSt4", bufs=1, space="PSUM"))
            lnp = ph4.enter_context(tc.tile_pool(name="lnp4", bufs=2))

            hT_sb = hpool.tile([128, NFC, TOK], BF16)
            y2_sb = hpool.tile([128, NDC, TOK], F32)
            sq2_sb = hpool.tile([128, NDC, TOK], F32)
            out_sb = hpool.tile([128, NDC, TOK], F32)

            for fc in range(NFC):
                w1_t = w1pool.tile([128, NDC, 128], BF16, tag="w1")
                nc.sync.dma_start(
                    out=w1_t, in_=_dram_chunked(w1, DFF)[:, :, fc * 128:(fc + 1) * 128])
                ps = psF.tile([128, TOK], F32, tag="f")
                for dc in range(NDC):
                    nc.tensor.matmul(ps, w1_t[:, dc, :], ln1b_sb[:, dc, :],
                                     start=(dc == 0), stop=(dc == NDC - 1))
                nc.scalar.activation(out=hT_sb[:, fc, :], in_=ps, func=AF.Relu,
                                     bias=b1_sb[:, fc:fc + 1], scale=1.0)

            for mc in range(NDC):
                w2_t = w2pool.tile([128, NFC, 128], BF16, tag="w2")
                nc.sync.dma_start(
                    out=w2_t, in_=_dram_chunked(w2, D)[:, :, mc * 128:(mc + 1) * 128])
                ps = psF2.tile([128, TOK], F32, tag="f2")
                for fc in range(NFC):
                    nc.tensor.matmul(ps, w2_t[:, fc, :], hT_sb[:, fc, :],
                                     start=(fc == 0), stop=(fc == NFC - 1))
                nc.vector.scalar_tensor_tensor(out=y2_sb[:, mc, :], in0=ps,
                                               scalar=b2_sb[:, mc:mc + 1],
                                               in1=ln1_sb[:, mc, :],
                                               op0=ALU.add, op1=ALU.add)

            _layernorm(nc, tc, psSt, lnp, y2_sb, sq2_sb, ones_c, ones_r, eps_t,
                       g2_sb, be2_sb, out_sb, None,
                       out_dma=(_dram_chunked(outT, TOK), nc))

    nc.compile()
    return nc


def _layernorm(nc, tc, psSt, lnp, y_sb, sq_sb, ones_c, ones_r, eps_t,
               g_sb, be_sb, out_f32, out_bf16, out_dma=None):
    """LayerNorm over the feature (partition x chunk) axis of y_sb
    [128, NDC, TOK].  Stats via fp32 ones-matmuls; result written to
    out_f32 (fp32) and optionally out_bf16 (bf16)."""
    # sum and sum-of-squares over all 1024 features, per token
    for mc in range(NDC):
        nc.scalar.square(out=sq_sb[:, mc, :], in_=y_sb[:, mc, :])
    s_ps = psSt.tile([1, TOK], F32, tag="sum")
    q_ps = psSt.tile([1, TOK], F32, tag="sq")
    for mc in range(NDC):
        nc.tensor.matmul(s_ps, ones_c, y_sb[:, mc, :],
                         start=(mc == 0), stop=(mc == NDC - 1))
        nc.tensor.matmul(q_ps, ones_c, sq_sb[:, mc, :],
                         start=(mc == 0), stop=(mc == NDC - 1))
    mean_t = lnp.tile([1, TOK], F32, tag="mean")
    nc.scalar.mul(out=mean_t, in_=s_ps, mul=1.0 / D)
    msq_t = lnp.tile([1, TOK], F32, tag="msq")
    nc.scalar.mul(out=msq_t, in_=q_ps, mul=1.0 / D)
    var_t = lnp.tile([1, TOK], F32, tag="var")
    nc.vector.tensor_tensor(out=var_t, in0=mean_t, in1=mean_t, op=ALU.mult)
    nc.vector.tensor_sub(out=var_t, in0=msq_t, in1=var_t)
    std_t = lnp.tile([1, TOK], F32, tag="std")
    nc.scalar.activation(out=std_t, in_=var_t, func=AF.Sqrt, bias=eps_t, scale=1.0)
    rstd_t = lnp.tile([1, TOK], F32, tag="rstd")
    nc.vector.reciprocal_approx_fast(out=rstd_t, in_=std_t)
    # broadcast mean and rstd across 128 partitions (DMA partition-broadcast)
    mb_t = lnp.tile([128, TOK], F32, tag="mbt")
    nc.gpsimd.partition_broadcast(mb_t[:], mean_t[:], channels=128)
    rb_t = lnp.tile([128, TOK], F32, tag="rbt")
    nc.gpsimd.partition_broadcast(rb_t[:], rstd_t[:], channels=128)
    for mc in range(NDC):
        t1 = lnp.tile([128, TOK], F32, tag="t1")
        nc.vector.tensor_sub(out=t1, in0=y_sb[:, mc, :], in1=mb_t)
        t2 = lnp.tile([128, TOK], F32, tag="t2")
        # (y - mean) * g * rstd
        nc.vector.scalar_tensor_tensor(out=t2, in0=t1, scalar=g_sb[:, mc:mc + 1],
                                       in1=rb_t, op0=ALU.mult, op1=ALU.mult)
        nc.vector.tensor_scalar(out=out_f32[:, mc, :], in0=t2,
                                scalar1=be_sb[:, mc:mc + 1], scalar2=None,
                                op0=ALU.add)
        if out_dma is not None:
            dram, _nc = out_dma
            _nc.sync.dma_start(out=dram[:, mc, :], in_=out_f32[:, mc, :])
        if out_bf16 is not None:
            nc.vector.tensor_scalar(out=out_bf16[:, mc, :], in0=t2,
                                    scalar1=be_sb[:, mc:mc + 1], scalar2=None,
                                    op0=ALU.add)


_COMPILED = None
_LAST_IN_MAPS = None


def kernel(**inputs):
    global _COMPILED, _LAST_IN_MAPS
    ins = {k: np.asarray(v) for k, v in inputs.items()}
    x = _f32(ins["x"])
    Wq, bq = ins["Wq"], ins["bq"]
    Wk, bk = ins["Wk"], ins["bk"]
    Wv, bv = ins["Wv"], ins["bv"]
    Wo, bo = ins["Wo"], ins["bo"]
    W1, b1 = ins["W1"], ins["b1"]
    W2, b2 = ins["W2"], ins["b2"]
    g1, be1 = ins["g1"], ins["be1"]
    g2, be2 = ins["g2"], ins["be2"]

    wqkv = np.concatenate(
        [Wq.transpose(1, 0, 2).reshape(D, H * DK),
         Wk.transpose(1, 0, 2).reshape(D, H * DK),
         Wv.transpose(1, 0, 2).reshape(D, H * DV)], axis=1)
    bqkv = np.concatenate([bq.reshape(-1), bk.reshape(-1), bv.reshape(-1)])

    shared = {
        "wqkv": _bf(wqkv),
        "bqkv": _f32(bqkv.reshape(3 * D, 1)),
        "bv_row": _bf(bv.reshape(1, H * DV)),
        "wo": _bf(Wo),
        "bo": _f32(bo.reshape(D, 1)),
        "w1": _bf(W1),
        "b1": _f32(b1.reshape(DFF, 1)),
        "w2": _bf(W2),
        "b2": _f32(b2.reshape(D, 1)),
        "g1": _f32(g1.reshape(D, 1)),
        "be1": _f32(be1.reshape(D, 1)),
        "g2": _f32(g2.reshape(D, 1)),
        "be2": _f32(be2.reshape(D, 1)),
    }

    in_maps = []
    for c in range(NCORES):
        b, qoff = c // 4, (c % 4) * TOK
        xb = x[b]                        # (S, D) fp32
        xkT = np.ascontiguousarray(xb.T)         # (D, S)
        xqT = np.ascontiguousarray(xb[qoff:qoff + TOK].T)  # (D, TOK)
        m = dict(shared)
        m["xkT"] = _bf(xkT)
        m["xqT"] = _bf(xqT)
        m["xqTf"] = _f32(xqT)
        in_maps.append(m)
    _LAST_IN_MAPS = in_maps

    if _COMPILED is None:
        _COMPILED = build()
    res = bass_utils.run_bass_kernel_spmd(_COMPILED, in_maps,
                                          core_ids=list(range(NCORES)))
    out = np.empty((B, S, D), np.float32)
    for c in range(NCORES):
        b, qoff = c // 4, (c % 4) * TOK
        out[b, qoff:qoff + TOK, :] = res.results[c]["outT"].T
    return out

